# revision 32
# baseline (speedup 1.0000x reference)
"""MoE layer (8 experts, top-2, shared expert) on 8 Trainium2 NeuronCores.

Strategy: expert-parallel. Every core receives the full token set, computes the
router (fp32) redundantly, gathers the tokens routed to ITS expert (capacity
640 of 2048*2/8=512 avg), runs the expert FFN in float32r, scatters weighted
outputs into a [T,H] partial buffer, and a ReduceScatter sums partials and
hands each core its 256-token output shard.  The shared expert is data-parallel
(each core computes its own 256-token slice) and added after the RS.
"""
import numpy as np

import concourse.bass as bass
import concourse.bacc as bacc
import concourse.mybir as mybir
import concourse.tile as tile
from concourse.bass import IndirectOffsetOnAxis
from concourse.bass_utils import run_bass_kernel_spmd
from concourse.masks import make_identity, make_upper_triangular

F32 = mybir.dt.float32
F32R = mybir.dt.float32r
I32 = mybir.dt.int32
AF = mybir.ActivationFunctionType
OP = mybir.AluOpType

N_CORES = 8
B, S, H = 4, 512, 1024
T = B * S                # 2048 tokens
I = 2816                 # expert intermediate
IS = 1408                # shared intermediate
E = 8
CAP = 640                # per-expert token capacity (max observed ~551; 640 = +6 sigma)
NT = T // 128            # 16 token tiles
NH = H // 128            # 8 hidden chunks
NI = I // 128            # 22 intermediate chunks
NIS = IS // 128          # 11 shared intermediate chunks
NC = CAP // 128          # 5 capacity chunks
TS = T // N_CORES        # 256 tokens per core (shared expert / output shard)

_cached = {}
DEBUG = False


def build():
    nc = bacc.Bacc("TRN2", target_bir_lowering=False, debug=False, num_devices=N_CORES)

    # ---- per-core external inputs ----
    x = nc.dram_tensor("x", [T, H], F32R, kind="ExternalInput")        # gather source
    xt = nc.dram_tensor("xt", [H, T], F32, kind="ExternalInput")       # x^T for router
    gw = nc.dram_tensor("gw", [H, E], F32, kind="ExternalInput")
    wg = nc.dram_tensor("wg", [H, I], F32R, kind="ExternalInput")
    wu = nc.dram_tensor("wu", [H, I], F32R, kind="ExternalInput")
    wd = nc.dram_tensor("wd", [I, H], F32R, kind="ExternalInput")
    sg = nc.dram_tensor("sg", [H, IS], F32R, kind="ExternalInput")
    su = nc.dram_tensor("su", [H, IS], F32R, kind="ExternalInput")
    sd = nc.dram_tensor("sd", [IS, H], F32R, kind="ExternalInput")
    xst = nc.dram_tensor("xst", [H, TS], F32R, kind="ExternalInput")   # this core's token slice, transposed
    sel = nc.dram_tensor("sel", [128, E], F32, kind="ExternalInput")   # one-hot row of this core's expert
    out = nc.dram_tensor("out", [TS, H], F32, kind="ExternalOutput")
    if DEBUG:
        d_logits = nc.dram_tensor("d_logits", [128, NT, E], F32, kind="ExternalOutput")
        d_wc = nc.dram_tensor("d_wc", [128, NT], F32, kind="ExternalOutput")
        d_mask = nc.dram_tensor("d_mask", [128, NT], F32, kind="ExternalOutput")
        d_pos = nc.dram_tensor("d_pos", [128, NT], F32, kind="ExternalOutput")
        d_slot = nc.dram_tensor("d_slot", [128, NT], F32, kind="ExternalOutput")
        d_tok = nc.dram_tensor("d_tok", [128, NC], I32, kind="ExternalOutput")
        d_dst = nc.dram_tensor("d_dst", [128, NC], I32, kind="ExternalOutput")
        d_w = nc.dram_tensor("d_w", [128, NC], F32, kind="ExternalOutput")
        d_xgt0 = nc.dram_tensor("d_xgt0", [128, CAP], F32, kind="ExternalOutput")
        d_act0 = nc.dram_tensor("d_act0", [128, CAP], F32, kind="ExternalOutput")

    # ---- internal DRAM ----
    partial0 = nc.dram_tensor("partial0", [T + 1, 512], F32)  # weighted expert outputs, cols 0:512
    partial1 = nc.dram_tensor("partial1", [T + 1, 512], F32)  # cols 512:1024
    rs0 = nc.dram_tensor("rs0", [TS, 512], F32)
    rs1 = nc.dram_tensor("rs1", [TS, 512], F32)

    with tile.TileContext(nc) as tc:
        with (
            tc.tile_pool(name="const", bufs=1) as cpool,
            tc.tile_pool(name="route", bufs=1) as rpool,
            tc.tile_pool(name="xtp", bufs=2) as xtpool,
            tc.tile_pool(name="xgp", bufs=2) as xgpool,
            tc.tile_pool(name="xgt", bufs=1) as xgtpool,
            tc.tile_pool(name="acts", bufs=1) as actpool,
            tc.tile_pool(name="wgu", bufs=2) as wgupool,
            tc.tile_pool(name="wdp", bufs=5) as wdpool,
            tc.tile_pool(name="sdp", bufs=1) as sdpool,
            tc.tile_pool(name="ev", bufs=2) as evpool,
            tc.tile_pool(name="dop", bufs=1) as dopool,
        ):
            ps_phase_a = tc.tile_pool(name="ps_small", bufs=1, space="PSUM")
            ps_sm = ps_phase_a.__enter__()
            ps_phase_tr = tc.tile_pool(name="ps_tr", bufs=2, space="PSUM")
            ps_tr = ps_phase_tr.__enter__()
            # ================= constants =================
            ident_f = cpool.tile([128, 128], F32)
            make_identity(nc, ident_f[:])
            ident_rt = cpool.tile([128, 128], F32R)
            nc.vector.tensor_copy(ident_rt[:], ident_f[:])
            ident_r = ident_rt[:]
            u128 = cpool.tile([128, 128], F32)
            make_upper_triangular(nc, u128[:], 1.0, diag=False)   # u128[k,m]=1 iff k<m
            u16 = cpool.tile([16, 16], F32)
            make_upper_triangular(nc, u16[:], 1.0, diag=False)
            ones128 = cpool.tile([128, 1], F32)
            nc.vector.memset(ones128[:], 1.0)
            gw_sb = cpool.tile([128, NH, E], F32)
            nc.sync.dma_start(gw_sb[:], gw.rearrange("(hc p) e -> p hc e", p=128))
            sel_sb = cpool.tile([128, E], F32)
            nc.sync.dma_start(sel_sb[:], sel[:])
            ids_int = cpool.tile([128, NT], I32)
            nc.gpsimd.iota(ids_int[:], pattern=[[128, NT]], base=0, channel_multiplier=1)
            zrow = cpool.tile([128, 512], F32)
            nc.vector.memset(zrow[:], 0.0)

            iota_sf = cpool.tile([128, CAP], F32)
            nc.gpsimd.iota(iota_sf[:], pattern=[[1, CAP]], base=0, channel_multiplier=0,
                           allow_small_or_imprecise_dtypes=True)

            # ================= router: logits = x @ gw  (fp32) =================
            logits = rpool.tile([128, NT, E], F32)
            for tp in range(NT // 2):
                xt_t = xtpool.tile([128, NH, 256], F32, tag="xt")
                nc.scalar.dma_start(
                    xt_t[:], xt[:, tp * 256:(tp + 1) * 256].rearrange("(hc p) t -> p hc t", p=128)
                )
                for sub in range(2):
                    t = tp * 2 + sub
                    ps = ps_sm.tile([128, E], F32, tag="sm")
                    for h in range(NH):
                        nc.tensor.matmul(ps[:], xt_t[:, h, sub * 128:(sub + 1) * 128],
                                         gw_sb[:, h, :], start=(h == 0), stop=(h == NH - 1))
                    nc.vector.tensor_copy(logits[:, t, :], ps[:])

            # zero the partial buffers (T+1 rows each) — gpsimd queue, off the
            # sync queue that feeds the router/weight streams
            for r in range(T // 128):
                nc.gpsimd.dma_start(partial0[r * 128:(r + 1) * 128, :], zrow[:])
                nc.gpsimd.dma_start(partial1[r * 128:(r + 1) * 128, :], zrow[:])
            nc.gpsimd.dma_start(partial0[T:T + 1, :], zrow[0:1, :])
            nc.gpsimd.dma_start(partial1[T:T + 1, :], zrow[0:1, :])

            # ================= top-2, combine weights =================
            m8 = rpool.tile([128, NT, 8], F32)
            for t in range(NT):
                nc.vector.max(m8[:, t, :], logits[:, t, :])
            m1 = m8[:, :, 0:1]        # [128, NT, 1]
            m2 = m8[:, :, 1:2]
            pd = rpool.tile([128, NT], F32)
            nc.vector.tensor_tensor(pd[:], m8[:, :, 1], m8[:, :, 0], op=OP.subtract)
            p1 = rpool.tile([128, NT], F32)
            nc.scalar.activation(p1[:], pd[:], AF.Sigmoid, scale=-1.0)   # sigmoid(m1-m2)
            # eq masks vs broadcast m1/m2 over expert dim
            eq = rpool.tile([128, NT, E], F32)
            s1 = rpool.tile([128, NT], F32)
            s2 = rpool.tile([128, NT], F32)
            selb = rpool.tile([128, NT, E], F32)
            nc.vector.tensor_copy(selb[:], sel_sb[:].rearrange("p (o e) -> p o e", o=1)
                                  .to_broadcast([128, NT, E]))
            nc.vector.tensor_tensor(eq[:], logits[:], m1.to_broadcast([128, NT, E]), op=OP.is_equal)
            nc.vector.tensor_tensor(eq[:], eq[:], selb[:], op=OP.mult)
            nc.vector.reduce_sum(s1[:], eq[:], axis=mybir.AxisListType.X)
            nc.vector.tensor_tensor(eq[:], logits[:], m2.to_broadcast([128, NT, E]), op=OP.is_equal)
            nc.vector.tensor_tensor(eq[:], eq[:], selb[:], op=OP.mult)
            nc.vector.reduce_sum(s2[:], eq[:], axis=mybir.AxisListType.X)
            # wc = s1*p1 + s2*(1-p1);  mask01 = s1 + s2
            wc = rpool.tile([128, NT], F32)
            tmp = rpool.tile([128, NT], F32)
            nc.vector.tensor_tensor(wc[:], s1[:], p1[:], op=OP.mult)
            nc.vector.tensor_scalar(tmp[:], p1[:], -1.0, 1.0, op0=OP.mult, op1=OP.add)  # 1-p1
            nc.vector.tensor_tensor(tmp[:], s2[:], tmp[:], op=OP.mult)
            nc.vector.tensor_tensor(wc[:], wc[:], tmp[:], op=OP.add)
            mask01 = rpool.tile([128, NT], F32)
            nc.vector.tensor_tensor(mask01[:], s1[:], s2[:], op=OP.add)

            # ================= dispatch positions (cumsum) =================
            ps_cum = ps_sm.tile([128, NT], F32, tag="sm")
            nc.tensor.matmul(ps_cum[:], u128[:], mask01[:], start=True, stop=True)
            excl = rpool.tile([128, NT], F32)
            nc.vector.tensor_copy(excl[:], ps_cum[:])
            # column sums -> [NT, 1] via matmul with ones
            ps_cs = ps_sm.tile([NT, 1], F32, tag="sm")
            nc.tensor.matmul(ps_cs[:], mask01[:], ones128[:], start=True, stop=True)
            colsT = rpool.tile([NT, 1], F32)
            nc.vector.tensor_copy(colsT[:], ps_cs[:])
            colsTb = rpool.tile([NT, 128], F32)
            nc.vector.tensor_copy(colsTb[:], colsT[:].to_broadcast([NT, 128]))
            ps_off = ps_sm.tile([128, NT], F32, tag="sm")
            nc.tensor.matmul(ps_off[:], colsTb[:], u16[:], start=True, stop=True)
            pos = rpool.tile([128, NT], F32)
            nc.vector.tensor_tensor(pos[:], excl[:], ps_off[:], op=OP.add)
            # slot = mask ? min(pos, CAP) : CAP
            slot_f = rpool.tile([128, NT], F32)
            nc.vector.tensor_scalar_add(slot_f[:], pos[:], -float(CAP))
            nc.vector.tensor_tensor(slot_f[:], slot_f[:], mask01[:], op=OP.mult)
            nc.vector.tensor_scalar(slot_f[:], slot_f[:], float(CAP), float(CAP),
                                    op0=OP.add, op1=OP.min)
            slot_i = rpool.tile([128, NT], I32)
            nc.vector.tensor_copy(slot_i[:], slot_f[:])

            # build slot maps on-chip: maps[s, :] = P^T @ [ids, wc, ones] where
            # P[t, s] = (slot[t] == s).  One MM chain per 128-slot chunk.
            rhs3 = rpool.tile([128, NT, 3], F32)
            nc.vector.tensor_copy(rhs3[:, :, 0], ids_int[:])
            nc.vector.tensor_copy(rhs3[:, :, 1], wc[:])
            nc.vector.memset(rhs3[:, :, 2], 1.0)
            maps = rpool.tile([128, NC, 3], F32)
            for m in range(NC):
                ps3 = ps_sm.tile([128, 3], F32, tag="sm")
                for t in range(NT):
                    p_t = xgpool.tile([128, 128], F32, tag="pt")
                    nc.vector.tensor_scalar(p_t[:], iota_sf[:, m * 128:(m + 1) * 128],
                                            slot_f[:, t:t + 1], None, op0=OP.is_equal)
                    nc.tensor.matmul(ps3[:], p_t[:], rhs3[:, t, :],
                                     start=(t == 0), stop=(t == NT - 1))
                nc.vector.tensor_copy(maps[:, m, :], ps3[:])
            tok_sb = rpool.tile([128, NC], I32)
            dst_sb = rpool.tile([128, NC], I32)
            w_sb = rpool.tile([128, NC], F32)
            dst_f = rpool.tile([128, NC], F32)
            nc.vector.tensor_copy(tok_sb[:], maps[:, :, 0])
            nc.vector.tensor_copy(w_sb[:], maps[:, :, 1])
            # dst = tok + (1-used)*T  (unused slots -> trash row T)
            nc.vector.tensor_scalar(dst_f[:], maps[:, :, 2], -float(T), float(T),
                                    op0=OP.mult, op1=OP.add)
            nc.vector.tensor_tensor(dst_f[:], dst_f[:], maps[:, :, 0], op=OP.add)
            nc.vector.tensor_copy(dst_sb[:], dst_f[:])

            if DEBUG:
                nc.sync.dma_start(d_logits[:], logits[:])
                nc.sync.dma_start(d_wc[:], wc[:])
                nc.sync.dma_start(d_mask[:], mask01[:])
                nc.sync.dma_start(d_pos[:], pos[:])
                nc.sync.dma_start(d_slot[:], slot_f[:])
                nc.sync.dma_start(d_tok[:], tok_sb[:])
                nc.sync.dma_start(d_dst[:], dst_sb[:])
                nc.sync.dma_start(d_w[:], w_sb[:])

            # ================= gather + transpose -> xgt[h] [128, CAP] =================
            xgt = [xgtpool.tile([128, CAP], F32R, tag=f"xgt{h}", name=f"xgt{h}") for h in range(NH)]
            for j in range(NC):
                xg = xgpool.tile([128, H], F32R, tag="xg")
                nc.gpsimd.indirect_dma_start(
                    out=xg[:], out_offset=None,
                    in_=x[:], in_offset=IndirectOffsetOnAxis(ap=tok_sb[:, j:j + 1], axis=0))
                for h in range(NH):
                    pt = ps_tr.tile([128, 128], F32R, tag="tr")
                    nc.tensor.transpose(pt[:], xg[:, h * 128:(h + 1) * 128], ident_r)
                    nc.vector.tensor_copy(xgt[h][:, j * 128:(j + 1) * 128], pt[:])

            if DEBUG:
                nc.sync.dma_start(d_xgt0[:], xgt[0][:].bitcast(F32))
            ps_phase_tr.__exit__(None, None, None)
            ps_phase_a.__exit__(None, None, None)
            ps_phase_b = tc.tile_pool(name="ps_gu", bufs=2, space="PSUM")
            ps_gu = ps_phase_b.__enter__()

            # ================= expert FFN: gate/up =================
            acts = [actpool.tile([128, CAP], F32R, tag=f"act{i}", name=f"act{i}") for i in range(NI)]
            NSPLIT = [(0, 512), (512, CAP)]
            for i in range(NI):
                if i % 2 == 0:
                    wg_t = wgupool.tile([128, NH, 256], F32R, tag="wg")
                    nc.scalar.dma_start(wg_t[:], wg[:, i * 128:(i + 2) * 128]
                                        .rearrange("(hc p) i -> p hc i", p=128))
                    wu_t = wgupool.tile([128, NH, 256], F32R, tag="wu")
                    nc.scalar.dma_start(wu_t[:], wu[:, i * 128:(i + 2) * 128]
                                        .rearrange("(hc p) i -> p hc i", p=128))
                io = (i % 2) * 128
                g_psA = ps_gu.tile([128, 384], F32, tag="gu_gA")
                g_psB = ps_gu.tile([128, 256], F32, tag="gu_gB")
                u_psA = ps_gu.tile([128, 384], F32, tag="gu_uA")
                u_psB = ps_gu.tile([128, 256], F32, tag="gu_uB")
                for h in range(NH):
                    nc.tensor.matmul(g_psA[:], wg_t[:, h, io:io + 128], xgt[h][:, 0:384],
                                     start=(h == 0), stop=(h == NH - 1))
                    nc.tensor.matmul(g_psB[:], wg_t[:, h, io:io + 128], xgt[h][:, 384:CAP],
                                     start=(h == 0), stop=(h == NH - 1))
                    nc.tensor.matmul(u_psA[:], wu_t[:, h, io:io + 128], xgt[h][:, 0:384],
                                     start=(h == 0), stop=(h == NH - 1))
                    nc.tensor.matmul(u_psB[:], wu_t[:, h, io:io + 128], xgt[h][:, 384:CAP],
                                     start=(h == 0), stop=(h == NH - 1))
                nc.scalar.activation(acts[i][:, 0:384], g_psA[:], AF.Silu)
                nc.scalar.activation(acts[i][:, 384:CAP], g_psB[:], AF.Silu)
                nc.vector.tensor_tensor(acts[i][:, 0:384], acts[i][:, 0:384], u_psA[:], op=OP.mult)
                nc.vector.tensor_tensor(acts[i][:, 384:CAP], acts[i][:, 384:CAP], u_psB[:], op=OP.mult)

            if DEBUG:
                nc.sync.dma_start(d_act0[:], acts[0][:].bitcast(F32))

            # ================= shared expert: gate/up =================
            xst_sb = cpool.tile([128, NH, TS], F32R)
            nc.sync.dma_start(xst_sb[:], xst.rearrange("(hc p) t -> p hc t", p=128))
            sacts = [actpool.tile([128, TS], F32R, tag=f"sact{i}", name=f"sact{i}") for i in range(NIS)]
            for i in range(NIS):
                sg_w = sdpool.tile([128, NH, 128], F32R, tag="sgw")
                nc.sync.dma_start(sg_w[:], sg[:, i * 128:(i + 1) * 128]
                                  .rearrange("(hc p) i -> p hc i", p=128))
                su_w = sdpool.tile([128, NH, 128], F32R, tag="suw")
                nc.sync.dma_start(su_w[:], su[:, i * 128:(i + 1) * 128]
                                  .rearrange("(hc p) i -> p hc i", p=128))
                so = 0
                g_ps = ps_gu.tile([128, TS], F32, tag="gu_gB")
                u_ps = ps_gu.tile([128, TS], F32, tag="gu_uB")
                for h in range(NH):
                    nc.tensor.matmul(g_ps[:], sg_w[:, h, so:so + 128], xst_sb[:, h, :],
                                     start=(h == 0), stop=(h == NH - 1))
                    nc.tensor.matmul(u_ps[:], su_w[:, h, so:so + 128], xst_sb[:, h, :],
                                     start=(h == 0), stop=(h == NH - 1))
                nc.scalar.activation(sacts[i][:], g_ps[:], AF.Silu)
                nc.vector.tensor_tensor(sacts[i][:], sacts[i][:], u_ps[:], op=OP.mult)

            ps_phase_b.__exit__(None, None, None)
            ps_phase_c = tc.tile_pool(name="ps_dd", bufs=1, space="PSUM")
            ps_dd = ps_phase_c.__enter__()

            # ================= expert down proj + weighted scatter =================
            # (scatter full 1024-wide rows: walrus derives the dynamic-AP row
            #  stride from the out AP's shape, so out must be the full tensor)
            for nh_i, (a, b) in enumerate([(0, 512), (512, 1024)]):
                part = partial0 if nh_i == 0 else partial1
                for i in range(NI):
                    wd_t = wdpool.tile([128, 512], F32R, tag="wd")
                    nc.sync.dma_start(wd_t[:], wd[i * 128:(i + 1) * 128, a:b])
                    for m in range(NC):
                        dd = ps_dd.tile([128, 512], F32, tag=f"dd{m}")
                        nc.tensor.matmul(dd[:], acts[i][:, m * 128:(m + 1) * 128], wd_t[:],
                                         start=(i == 0), stop=(i == NI - 1))
                        if i == NI - 1:
                            o = dopool.tile([128, 512], F32, tag="dout", bufs=2)
                            nc.vector.tensor_tensor(
                                o[:], dd[:],
                                w_sb[:, m:m + 1].to_broadcast([128, 512]), op=OP.mult)
                            nc.gpsimd.indirect_dma_start(
                                out=part[:],
                                out_offset=IndirectOffsetOnAxis(ap=dst_sb[:, m:m + 1], axis=0),
                                in_=o[:], in_offset=None)
                if nh_i == 0:
                    nc.gpsimd.collective_compute(
                        "ReduceScatter", OP.add,
                        ins=[partial0[0:T, :]], outs=[rs0[:]],
                        replica_groups=[list(range(N_CORES))],
                    )

            # ================= combine: second ReduceScatter + shared add =================
            nc.gpsimd.collective_compute(
                "ReduceScatter", OP.add,
                ins=[partial1[0:T, :]], outs=[rs1[:]],
                replica_groups=[list(range(N_CORES))],
            )
            # ================= shared down proj =================
            sh_out = cpool.tile([128, 2, H], F32)
            for m in range(2):
                sdd0 = ps_dd.tile([128, 512], F32, tag="sdd0")
                sdd1 = ps_dd.tile([128, 512], F32, tag="sdd1")
                for i in range(NIS):
                    sd_a = sdpool.tile([128, 512], F32R, tag="sd_a")
                    nc.sync.dma_start(sd_a[:], sd[i * 128:(i + 1) * 128, 0:512])
                    sd_b = sdpool.tile([128, 512], F32R, tag="sd_b")
                    nc.sync.dma_start(sd_b[:], sd[i * 128:(i + 1) * 128, 512:1024])
                    nc.tensor.matmul(sdd0[:], sacts[i][:, m * 128:(m + 1) * 128],
                                     sd_a[:], start=(i == 0), stop=(i == NIS - 1))
                    nc.tensor.matmul(sdd1[:], sacts[i][:, m * 128:(m + 1) * 128],
                                     sd_b[:], start=(i == 0), stop=(i == NIS - 1))
                nc.vector.tensor_copy(sh_out[:, m, 0:512], sdd0[:])
                nc.vector.tensor_copy(sh_out[:, m, 512:1024], sdd1[:])

            ps_phase_c.__exit__(None, None, None)
            rs_sb = cpool.tile([128, 2, H], F32)
            nc.sync.dma_start(rs_sb[:, :, 0:512], rs0.rearrange("(m p) h -> p m h", p=128))
            nc.sync.dma_start(rs_sb[:, :, 512:1024], rs1.rearrange("(m p) h -> p m h", p=128))
            for m in range(2):
                for (a, b) in [(0, 512), (512, 1024)]:
                    fin = dopool.tile([128, 512], F32, tag="fin")
                    nc.vector.tensor_tensor(fin[:], rs_sb[:, m, a:b], sh_out[:, m, a:b], op=OP.add)
                    nc.sync.dma_start(out[m * 128:(m + 1) * 128, a:b], fin[:])

    nc.compile()
    return nc


def kernel(hidden_states, gate_w, Wg, Wu, Wd, Sg, Su, Sd):
    hidden_states = np.ascontiguousarray(np.asarray(hidden_states, dtype=np.float32))
    gate_w = np.ascontiguousarray(np.asarray(gate_w, dtype=np.float32))
    Wg = np.asarray(Wg, dtype=np.float32)
    Wu = np.asarray(Wu, dtype=np.float32)
    Wd = np.asarray(Wd, dtype=np.float32)
    Sg = np.ascontiguousarray(np.asarray(Sg, dtype=np.float32))
    Su = np.ascontiguousarray(np.asarray(Su, dtype=np.float32))
    Sd = np.ascontiguousarray(np.asarray(Sd, dtype=np.float32))

    x2d = np.ascontiguousarray(hidden_states.reshape(T, H))
    x2dT = np.ascontiguousarray(x2d.T)

    if "nc" not in _cached:
        _cached["nc"] = build()
    nc = _cached["nc"]

    in_maps = []
    for c in range(N_CORES):
        selv = np.zeros((128, E), np.float32)
        selv[:, c] = 1.0
        in_maps.append({
            "x": x2d,
            "xt": x2dT,
            "gw": gate_w,
            "wg": np.ascontiguousarray(Wg[c]),
            "wu": np.ascontiguousarray(Wu[c]),
            "wd": np.ascontiguousarray(Wd[c]),
            "sg": Sg, "su": Su, "sd": Sd,
            "xst": np.ascontiguousarray(x2dT[:, c * TS:(c + 1) * TS]),
            "sel": selv,
        })

    res = run_bass_kernel_spmd(nc, in_maps, core_ids=list(range(N_CORES)),
                               trace=_cached.get("trace", False))
    _cached["last_result"] = res
    full = np.concatenate([res.results[c]["out"] for c in range(N_CORES)], axis=0)
    return full.reshape(B, S, H)
